# revision 1
# baseline (speedup 1.0000x reference)
"""Trainium2 Bass kernel: vq_codebook / nn_Anchor.

Reference computation (per batch row b):
  xn = l2_normalize(x[b], axis=-1)                       # [N, D]
  sq = 1 + |a_c|^2 - 2 xn.a_c                            # [N, C]
  score = softmax(1/sqrt(sq), axis=C), zeroed at invalid rows
  attr = argmax_c score; index = mode of attr over valid rows
  feature[b] = sum_i xn[i] * score[i, index]             # [D]

Device strategy: data-parallel over B across 8 cores (4 batch slots per
core).  Rows >= mask[b] contribute nothing, so the graph is specialized
at build time to the actual per-batch valid-tile counts: batches are
sorted by ceil(mask/128), snake-assigned to (core, slot), and each slot
compiles max-over-cores tiles -- identical instruction streams on all
cores (SPMD-safe), ~40% less work than the dense loop.

Per group of up to 4 row-tiles (128 rows each):
  - DMA x fp32; r2 = sum x^2 via fused square+accumulate (alternating
    DVE scalar_tensor_tensor / ACT Square to balance engines)
  - inv = rsqrt(r2) = exp(-.5 ln r2) on ACT (all ACT functions forced
    into the one natural_log_exp_and_others table set: no reloads)
  - xn = x * inv cast to bf16 (GPSIMD tensor_scalar, freeing DVE)
  - one xbar DMA transpose for the whole group -> xnT [128d, 4H, 128]
  - mm1: sT[64, H*128] += at2n[k].T @ xnT[:, k::4, :] (anchor-stationary)
  - Ln(sT + anb) straight from PSUM with anb as per-partition bias;
    L = exp(-.5 ln) -> fp16, shipped for host mode-selection; one xbar
    transpose back to row layout; E = exp(L)
  - ssum; q = vmask/ssum; W2 = E*q bf16; mm2: F[64,512] += W2.T @ xn
Host: attr = argmax_c L, counts = bincount(attr[valid]), index =
argmax(counts), feature = F[index].
"""

import numpy as np
import ml_dtypes

import concourse.bass as bass
import concourse.bacc as bacc
import concourse.mybir as mybir
import concourse.tile as tile
from concourse import masks
from concourse.bass_utils import run_bass_kernel_spmd

B, N, D, C = 32, 4096, 512, 64
NCORES = 8
BPC = B // NCORES          # batch slots per core
P = 128                    # rows per tile (SBUF partitions)
T = N // P                 # 32 row-tiles per batch max
KC = D // P                # 4 contraction chunks of 128
HMAX = 8                   # tiles per group

f32 = mybir.dt.float32
bf16 = mybir.dt.bfloat16
f16 = mybir.dt.float16

Alu = mybir.AluOpType
Act = mybir.ActivationFunctionType

USE_GPSIMD_CAST = False

# Force Ln/Exp onto the combined activation-table set so ACT never
# reloads tables mid-kernel.
_orig_gat = bacc.get_activation_tables


def _gat_single_set(arch):
    t = _orig_gat(arch)
    out = {}
    for name, fns in t.items():
        if name != "natural_log_exp_and_others":
            fns = fns - {Act.Ln, Act.Exp}
        out[name] = fns
    return out


bacc.get_activation_tables = _gat_single_set


def build(S):
    """S: per-slot static tile counts (same on every core)."""
    S = tuple(int(s) for s in S)
    ncols = [s * P for s in S]
    offs = np.concatenate([[0], np.cumsum(ncols)]).astype(int)
    totl = int(offs[-1])

    nc = bacc.Bacc("TRN2", target_bir_lowering=False, debug=False,
                   num_devices=NCORES)

    x_d = nc.dram_tensor("x", [BPC, N, D], f32, kind="ExternalInput")
    at2n_d = nc.dram_tensor("at2n", [P, KC, C], bf16, kind="ExternalInput")
    anb_d = nc.dram_tensor("anb", [C, 1], f32, kind="ExternalInput")
    vmask_d = nc.dram_tensor("vmask", [BPC, P, T], f32, kind="ExternalInput")
    L_d = nc.dram_tensor("L_out", [C, totl], f16, kind="ExternalOutput")
    F_d = nc.dram_tensor("F_out", [BPC, C, D], f32, kind="ExternalOutput")

    with tile.TileContext(nc) as tc:
        with (
            tc.tile_pool(name="singles", bufs=1) as singles,
            tc.tile_pool(name="xf", bufs=4) as xf_pool,
            tc.tile_pool(name="xn", bufs=4) as xn_pool,
            tc.tile_pool(name="xb", bufs=4) as xb_pool,
            tc.tile_pool(name="xnt", bufs=4) as xnt_pool,
            tc.tile_pool(name="x2", bufs=4) as x2_pool,
            tc.tile_pool(name="lnt", bufs=4) as lnt_pool,
            tc.tile_pool(name="lt", bufs=4) as lt_pool,
            tc.tile_pool(name="lrow", bufs=3) as lrow_pool,
            tc.tile_pool(name="ebuf", bufs=4) as e_pool,
            tc.tile_pool(name="small", bufs=5) as small_pool,
            tc.tile_pool(name="w2", bufs=8) as w2_pool,
            tc.tile_pool(name="fsb", bufs=2) as f_pool,
            tc.tile_pool(name="ps_s", bufs=2, space=bass.MemorySpace.PSUM) as ps_s,
            tc.tile_pool(name="ps_f", bufs=2, space=bass.MemorySpace.PSUM) as ps_f,
            tc.tile_pool(name="ps_l", bufs=2, space=bass.MemorySpace.PSUM) as ps_l,
        ):
            at2n_sb = singles.tile([P, KC, C], bf16)
            nc.sync.dma_start(at2n_sb[:], at2n_d[:])
            anbT = singles.tile([C, 1], f32)
            nc.sync.dma_start(anbT[:], anb_d[:])
            ident = singles.tile([P, P], f16)
            masks.make_identity(nc, ident[:])

            groups = []
            for b in range(BPC):
                t0 = 0
                while t0 < S[b]:
                    H = min(HMAX, S[b] - t0)
                    groups.append((b, t0, H))
                    t0 += H

            gstate = {}
            bstate = {}
            tglobal = [0]

            def front(g):
                b, t0, H = g
                if t0 == 0:
                    vm = small_pool.tile([P, T], f32, tag="vmask")
                    nc.sync.dma_start(vm[:], vmask_d[b])
                    f_ps = ps_f.tile([P, D], f32)
                    bstate[b] = (vm, f_ps)
                xf = xf_pool.tile([P, HMAX, D], f32, tag="xf")
                xb = xb_pool.tile([P, HMAX, D], bf16, tag="xb")
                r2 = small_pool.tile([P, HMAX], f32, tag="r2")
                for h0 in range(0, H, 4):
                    hs = min(4, H - h0)
                    nc.gpsimd.dma_start(
                        xf[:, h0:h0 + hs, :],
                        x_d[b, (t0 + h0) * P:(t0 + h0 + hs) * P, :].rearrange(
                            "(h p) d -> p h d", p=P))
                for i in range(H):
                    # plain cast: 2x-mode copy, split between DVE and ACT
                    if (tglobal[0] + i) % 2 == 0:
                        nc.vector.tensor_copy(xb[:, i, :], xf[:, i, :])
                    else:
                        nc.scalar.copy(xb[:, i, :], xf[:, i, :])
                    # r2 = sum x^2 from bf16 (2x-mode on DVE)
                    x2 = x2_pool.tile([P, D], bf16)
                    nc.vector.scalar_tensor_tensor(
                        out=x2[:], in0=xb[:, i, :], scalar=1.0,
                        in1=xb[:, i, :], op0=Alu.mult, op1=Alu.mult,
                        accum_out=r2[:, i:i + 1])
                tglobal[0] += H
                # inv = rsqrt(r2) = exp(-0.5 ln r2)
                lr2 = small_pool.tile([P, HMAX], f32, tag="lr2")
                nc.scalar.activation(lr2[:, :H], r2[:, :H], Act.Ln)
                inv = small_pool.tile([P, HMAX], f32, tag="inv")
                nc.scalar.activation(inv[:, :H], lr2[:, :H], Act.Exp,
                                     scale=-0.5)
                # normalize: bf16 4x-mode tensor_scalar
                xn = xn_pool.tile([P, HMAX, D], bf16, tag="xn")
                for i in range(H):
                    nc.vector.tensor_scalar_mul(xn[:, i, :], xb[:, i, :],
                                                inv[:, i:i + 1])
                # one xbar transpose for the whole group
                xnt = xnt_pool.tile([P, HMAX * KC, P], bf16, tag="xnt")
                nc.sync.dma_start_transpose(
                    xnt[:, :H * KC, :],
                    xn[:, :H, :].rearrange("p h d -> p (h d)"))
                # sT[64, H*128] += at2n[k].T @ xnT[k]
                sT = ps_s.tile([C, HMAX * P], f32)
                for hc in range(0, H, 4):
                    hsz = min(4, H - hc)
                    for k in range(KC):
                        nc.tensor.matmul(
                            sT[:, hc * P:(hc + hsz) * P],
                            at2n_sb[:, k, :],
                            xnt[:, hc * KC + k:(hc + hsz) * KC:KC, :],
                            start=(k == 0), stop=(k == KC - 1))
                gstate[g] = (xn, sT)

            def back(g):
                b, t0, H = g
                vm, f_ps = bstate[b]
                xn, sT = gstate.pop(g)
                lnT = lnt_pool.tile([C, HMAX * P], f32)
                nc.scalar.activation(lnT[:, :H * P], sT[:, :H * P], Act.Ln,
                                     bias=anbT[:])
                LT = lt_pool.tile([C, HMAX * P], f16)
                nc.scalar.activation(LT[:, :H * P], lnT[:, :H * P], Act.Exp,
                                     scale=-0.5)
                col0 = int(offs[b]) + t0 * P
                nc.gpsimd.dma_start(L_d[:, col0:col0 + H * P], LT[:, :H * P])
                ltr = ps_l.tile([P, HMAX, C], f16)
                for i in range(H):
                    nc.tensor.transpose(ltr[:, i, :],
                                        LT[:, i * P:(i + 1) * P],
                                        ident[:C, :C])
                Et = e_pool.tile([P, HMAX, C], bf16)
                nc.scalar.activation(Et[:, :H, :], ltr[:, :H, :], Act.Exp)
                ssum = small_pool.tile([P, HMAX], f32, tag="ssum")
                nc.vector.tensor_reduce(ssum[:, :H], Et[:, :H, :],
                                        axis=mybir.AxisListType.X, op=Alu.add)
                rs = small_pool.tile([P, HMAX], f32, tag="rs")
                nc.vector.reciprocal(rs[:, :H], ssum[:, :H])
                q2 = small_pool.tile([P, HMAX], f32, tag="q2")
                nc.vector.tensor_mul(q2[:, :H], rs[:, :H], vm[:, t0:t0 + H])
                for i in range(H):
                    t = t0 + i
                    w2 = w2_pool.tile([P, C], bf16)
                    nc.vector.tensor_scalar_mul(w2[:], Et[:, i, :],
                                                q2[:, i:i + 1])
                    nc.tensor.matmul(f_ps[C:, :], w2[:], xn[:, i, :],
                                     start=(t == 0),
                                     stop=(t == S[b] - 1),
                                     tile_position=(0, C))
                if t0 + H == S[b]:
                    fsb = f_pool.tile([C, D], f32)
                    nc.vector.tensor_copy(fsb[:], f_ps[C:, :])
                    nc.gpsimd.dma_start(F_d[b], fsb[:])

            SKEW = 1
            for gi, g in enumerate(groups):
                front(g)
                if gi >= SKEW:
                    back(groups[gi - SKEW])
            for g in groups[-SKEW:]:
                back(g)

    nc.compile()
    return nc


_CACHE = {}


def _plan(mask):
    """Sort batches by valid-tile count, snake-assign to (core, slot)."""
    tb = np.minimum((mask + P - 1) // P, T).astype(int)   # [B] tiles needed
    ranks = np.argsort(-tb, kind="stable")
    assign = np.empty((NCORES, BPC), dtype=int)
    S = []
    for j in range(BPC):
        block = ranks[j * NCORES:(j + 1) * NCORES]
        assign[:, j] = block
        S.append(int(tb[block].max()))
    return assign, tuple(S)


def _prep_in_maps(x, mask, anchors, assign):
    x = np.ascontiguousarray(np.asarray(x, dtype=np.float32))
    anchors = np.asarray(anchors, dtype=np.float32)

    a2 = (anchors.astype(np.float64) ** 2).sum(1)              # [C]
    anb = np.ascontiguousarray((1.0 + a2)[:, None]).astype(np.float32)
    atT = (-2.0 * anchors.T).astype(ml_dtypes.bfloat16)        # [D, C]
    at2n = np.ascontiguousarray(atT.reshape(KC, P, C).transpose(1, 0, 2))

    rows = np.arange(N)
    in_maps = []
    for c in range(NCORES):
        sel = assign[c]                                        # batch ids
        xb = x[sel]
        mb = mask[sel]
        vmv = rows[None, :] < mb[:, None]                      # [BPC, N]
        vmt = np.ascontiguousarray(
            vmv.reshape(BPC, T, P).transpose(0, 2, 1).astype(np.float32))
        in_maps.append({"x": np.ascontiguousarray(xb), "at2n": at2n,
                        "anb": anb, "vmask": vmt})
    return in_maps


def _postprocess(results, mask, assign, S):
    offs = np.concatenate([[0], np.cumsum([s * P for s in S])]).astype(int)
    feature = np.empty((B, D), dtype=np.float32)
    for c in range(NCORES):
        out = results[c]
        Lf = np.asarray(out["L_out"]).astype(np.float32)   # [C, totl]
        Ff = np.asarray(out["F_out"])                      # [BPC, C, D]
        for j in range(BPC):
            gb = int(assign[c, j])
            ncol = S[j] * P
            attr = Lf[:, offs[j]:offs[j] + ncol].argmax(axis=0)
            nvalid = int(mask[gb])
            counts = np.bincount(attr[:nvalid], minlength=C)
            idx = int(counts.argmax())
            feature[gb] = Ff[j, idx]
    return feature


def kernel(x, mask, anchors, _trace=False):
    mask = np.asarray(mask).astype(np.int64)
    assign, S = _plan(mask)
    if S not in _CACHE:
        _CACHE[S] = build(S)
    nc = _CACHE[S]
    in_maps = _prep_in_maps(x, mask, anchors, assign)
    res = run_bass_kernel_spmd(nc, in_maps, core_ids=list(range(NCORES)),
                               trace=_trace)
    feature = _postprocess(res.results, mask, assign, S)
    if _trace:
        return feature, res
    return feature

